# revision 2
# baseline (speedup 1.0000x reference)
"""AbsoluteLearnedPE logits kernel for one TRN2 chip (8 NeuronCores).

Reference math (B=8, Q=K=2048, D=1024, fp32):
    E = embed[:2048]                       # q_embed == k_embed == E
    out[b] = q[b] @ E^T + E @ (k[b] + E)^T

Sharding: data-parallel over batch — core b computes batch b. E is
replicated. Per core this is two [2048,1024]x[1024,2048] matmuls fused
into one 16-step PSUM accumulation (contraction 2*D = 2048).

On-chip pipeline per core:
  1. DMA e/k row-blocks, k+e add on DVE, PE-transpose (fp32, exact) each
     128x128 block of E and K+E into SBUF operand buffers; the PSUM->SBUF
     eviction rounds to fp32r (TensorE's full-rate 4-byte format).
  2. Per q row-block: DMA + transpose q likewise, then 64 fp32r matmuls
     (16 output tiles [128 x 512], 16-deep accumulation each), evict
     PSUM through DVE/ACT to SBUF, DMA to HBM.

fp32r matmul runs at ~1 cyc/row (bf16-rate) with ~1.5e-4 L2 relative
error vs fp32 — measured on HW.
"""

import numpy as np

B, Q, K, D = 8, 2048, 2048, 1024
DTILES = D // 128      # 8  contraction tiles per term
RBLK = Q // 128        # 16 row blocks of e/k
QT = Q // 128          # 16 q tiles
KSTRIPE = 512
KS = K // KSTRIPE      # 4  k stripes

_CACHE = {}
TRACE = False          # test harness can flip this for neuron-profile runs


def _build():
    from concourse import bacc
    import concourse.mybir as mybir
    import concourse.tile as tile
    from concourse.masks import make_identity

    f32 = mybir.dt.float32
    f32r = mybir.dt.float32r

    nc = bacc.Bacc("TRN2", target_bir_lowering=False, debug=False, num_devices=B)
    q = nc.dram_tensor("q", [Q, D], f32, kind="ExternalInput").ap()
    k = nc.dram_tensor("k", [K, D], f32, kind="ExternalInput").ap()
    e = nc.dram_tensor("e", [Q, D], f32, kind="ExternalInput").ap()
    out = nc.dram_tensor("out", [Q, K], f32, kind="ExternalOutput").ap()

    with tile.TileContext(nc) as tc:
        with tc.tile_pool(name="const", bufs=1) as const, \
             tc.tile_pool(name="nat", bufs=2) as nat, \
             tc.tile_pool(name="big", bufs=1) as big, \
             tc.tile_pool(name="qbuf", bufs=2) as qbuf, \
             tc.tile_pool(name="outp", bufs=3) as outp, \
             tc.tile_pool(name="tps", bufs=4, space="PSUM") as tps, \
             tc.tile_pool(name="mps", bufs=4, space="PSUM") as mps:

            ident = const.tile([128, 128], f32)
            make_identity(nc, ident)

            # Transposed operands, fp32r (rounded on PSUM->SBUF eviction).
            # eT[p, d, j]  = e[j, d*128+p];  keT[p, d, j] = (k+e)[j, d*128+p]
            eT = big.tile([128, DTILES, Q], f32r, tag="eT")
            keT = big.tile([128, DTILES, K], f32r, tag="keT")

            for r in range(RBLK):
                e_nat = nat.tile([128, D], f32, tag="e_nat")
                k_nat = nat.tile([128, D], f32, tag="k_nat")
                nc.sync.dma_start(out=e_nat[:], in_=e[r * 128:(r + 1) * 128, :])
                nc.sync.dma_start(out=k_nat[:], in_=k[r * 128:(r + 1) * 128, :])
                nc.vector.tensor_add(k_nat[:], k_nat[:], e_nat[:])
                for d in range(DTILES):
                    ps = tps.tile([128, 128], f32, tag="tps")
                    nc.tensor.transpose(ps[:], e_nat[:, d * 128:(d + 1) * 128], ident[:])
                    nc.any.tensor_copy(out=eT[:, d, r * 128:(r + 1) * 128], in_=ps[:])
                    ps2 = tps.tile([128, 128], f32, tag="tps")
                    nc.tensor.transpose(ps2[:], k_nat[:, d * 128:(d + 1) * 128], ident[:])
                    nc.any.tensor_copy(out=keT[:, d, r * 128:(r + 1) * 128], in_=ps2[:])

            for qt in range(QT):
                q_nat = nat.tile([128, D], f32, tag="q_nat")
                nc.sync.dma_start(out=q_nat[:], in_=q[qt * 128:(qt + 1) * 128, :])
                qT = qbuf.tile([128, DTILES, 128], f32r, tag="qT")
                for d in range(DTILES):
                    ps = tps.tile([128, 128], f32, tag="tps")
                    nc.tensor.transpose(ps[:], q_nat[:, d * 128:(d + 1) * 128], ident[:])
                    nc.any.tensor_copy(out=qT[:, d, :], in_=ps[:])
                for ks in range(KS):
                    pso = mps.tile([128, KSTRIPE], f32, tag="mps")
                    kslice = slice(ks * KSTRIPE, (ks + 1) * KSTRIPE)
                    for d in range(DTILES):
                        nc.tensor.matmul(pso[:], qT[:, d, :], eT[:, d, kslice],
                                         start=(d == 0), stop=False)
                    for d in range(DTILES):
                        nc.tensor.matmul(pso[:], eT[:, d, qt * 128:(qt + 1) * 128],
                                         keT[:, d, kslice],
                                         start=False, stop=(d == DTILES - 1))
                    o_t = outp.tile([128, KSTRIPE], f32, tag="o_t")
                    nc.any.tensor_copy(out=o_t[:], in_=pso[:])
                    nc.sync.dma_start(
                        out=out[qt * 128:(qt + 1) * 128, kslice], in_=o_t[:])
    nc.compile()
    return nc


def kernel(q: np.ndarray, k: np.ndarray, embed: np.ndarray) -> np.ndarray:
    from concourse.bass_utils import run_bass_kernel_spmd

    if "nc" not in _CACHE:
        _CACHE["nc"] = _build()
    nc = _CACHE["nc"]

    e = np.ascontiguousarray(embed[:Q], dtype=np.float32)
    in_maps = [
        {
            "q": np.ascontiguousarray(q[b], dtype=np.float32),
            "k": np.ascontiguousarray(k[b], dtype=np.float32),
            "e": e,
        }
        for b in range(B)
    ]
    res = run_bass_kernel_spmd(nc, in_maps, core_ids=list(range(B)), trace=TRACE)
    _CACHE["last_result"] = res
    return np.stack([res.results[b]["out"] for b in range(B)])


# revision 11
# speedup vs baseline: 1.0407x; 1.0407x over previous
"""AbsoluteLearnedPE logits kernel for one TRN2 chip (8 NeuronCores).

Reference math (B=8, Q=K=2048, D=1024, fp32):
    E = embed[:2048]                       # q_embed == k_embed == E
    out[b] = q[b] @ E^T + E @ (k[b] + E)^T

Sharding: data-parallel over batch — core b computes batch b. E is
replicated. Per core this is two [2048,1024]x[1024,2048] matmuls fused
into one 16-step PSUM accumulation (contraction 2*D = 2048).

On-chip pipeline per core:
  1. DMA e/k row-blocks, k+e add on DVE, PE-transpose (fp32, exact) each
     128x128 block of E and K+E into SBUF operand buffers; the PSUM->SBUF
     eviction rounds to fp32r (TensorE's full-rate 4-byte format).
  2. Per q row-block: DMA + transpose q likewise, then 64 fp32r matmuls
     (16 output tiles [128 x 512], 16-deep accumulation each), evict
     PSUM through DVE/ACT to SBUF, DMA to HBM.

fp32r matmul runs at ~1 cyc/row (bf16-rate) with ~1.5e-4 L2 relative
error vs fp32 — measured on HW.
"""

import numpy as np

B, Q, K, D = 8, 2048, 2048, 1024
DTILES = D // 128      # 8  contraction tiles per term
RBLK = Q // 128        # 16 row blocks of e/k
QT = Q // 128          # 16 q tiles
KSTRIPE = 512
KS = K // KSTRIPE      # 4  k stripes

_CACHE = {}
TRACE = False          # test harness can flip this for neuron-profile runs


def _build():
    from concourse import bacc
    import concourse.mybir as mybir
    import concourse.tile as tile
    from concourse.masks import make_identity

    f32 = mybir.dt.float32
    f32r = mybir.dt.float32r

    nc = bacc.Bacc("TRN2", target_bir_lowering=False, debug=False, num_devices=B)
    q = nc.dram_tensor("q", [Q, D], f32, kind="ExternalInput").ap()
    k = nc.dram_tensor("k", [K, D], f32, kind="ExternalInput").ap()
    e = nc.dram_tensor("e", [Q, D], f32, kind="ExternalInput").ap()
    out = nc.dram_tensor("out", [Q, K], f32, kind="ExternalOutput").ap()

    with tile.TileContext(nc) as tc:
        with tc.tile_pool(name="const", bufs=1) as const, \
             tc.tile_pool(name="nat", bufs=3) as nat, \
             tc.tile_pool(name="qnat", bufs=5) as qnat, \
             tc.tile_pool(name="big", bufs=1) as big, \
             tc.tile_pool(name="qbuf", bufs=2) as qbuf, \
             tc.tile_pool(name="outp", bufs=3) as outp, \
             tc.tile_pool(name="tps", bufs=4, space="PSUM") as tps, \
             tc.tile_pool(name="mps", bufs=4, space="PSUM") as mps:

            ident = const.tile([128, 128], f32)
            make_identity(nc, ident)

            # Transposed operands, fp32r (rounded on PSUM->SBUF eviction).
            # eT[p, d, j]  = e[j, d*128+p];  keT[p, d, j] = (k+e)[j, d*128+p]
            eT = big.tile([128, DTILES, Q], f32r, tag="eT")
            keT = big.tile([128, DTILES, K], f32r, tag="keT")

            # Interleave q / e / k row-block loads so the first matmul
            # group's operands (qT[0], eT/keT row-blocks 0-3) are ready
            # early and the PE never drains during the prologue. q blocks
            # are only DMA-prefetched here; their transposes are emitted in
            # the qt loop where each is consumed immediately (emitting them
            # here deadlocks: stalled qT slots starve the shared PSUM pool).
            QPRE = 5  # q-block DMA prefetch depth (== qnat bufs, never blocks)
            q_nats = []

            def prefetch_q(r):
                q_nat = qnat.tile([128, D], f32, tag="q_nat")
                q_nats.append(q_nat)
                nc.sync.dma_start(out=q_nat[:], in_=q[r * 128:(r + 1) * 128, :])

            for r in range(RBLK):
                if r < QPRE:
                    prefetch_q(r)
                e_nat = nat.tile([128, D], f32, tag="e_nat")
                k_nat = nat.tile([128, D], f32, tag="k_nat")
                nc.sync.dma_start(out=e_nat[:], in_=e[r * 128:(r + 1) * 128, :])
                nc.sync.dma_start(out=k_nat[:], in_=k[r * 128:(r + 1) * 128, :])
                nc.vector.tensor_add(k_nat[:], k_nat[:], e_nat[:])
                for d in range(DTILES):
                    ps = tps.tile([128, 128], f32, tag="tps")
                    nc.tensor.transpose(ps[:], e_nat[:, d * 128:(d + 1) * 128], ident[:])
                    nc.any.tensor_copy(out=eT[:, d, r * 128:(r + 1) * 128], in_=ps[:])
                    ps2 = tps.tile([128, 128], f32, tag="tps")
                    nc.tensor.transpose(ps2[:], k_nat[:, d * 128:(d + 1) * 128], ident[:])
                    nc.any.tensor_copy(out=keT[:, d, r * 128:(r + 1) * 128], in_=ps2[:])

            for qt in range(QT):
                if qt + QPRE < QT:
                    prefetch_q(qt + QPRE)
                q_nat = q_nats[qt]
                qT = qbuf.tile([128, DTILES, 128], f32r, tag="qT")
                for d in range(DTILES):
                    ps = tps.tile([128, 128], f32, tag="tps")
                    nc.tensor.transpose(ps[:], q_nat[:, d * 128:(d + 1) * 128], ident[:])
                    nc.any.tensor_copy(out=qT[:, d, :], in_=ps[:])
                for ks in range(KS):
                    pso = mps.tile([128, KSTRIPE], f32, tag="mps")
                    kslice = slice(ks * KSTRIPE, (ks + 1) * KSTRIPE)
                    for d in range(DTILES):
                        nc.tensor.matmul(pso[:], qT[:, d, :], eT[:, d, kslice],
                                         start=(d == 0), stop=False)
                    for d in range(DTILES):
                        nc.tensor.matmul(pso[:], eT[:, d, qt * 128:(qt + 1) * 128],
                                         keT[:, d, kslice],
                                         start=False, stop=(d == DTILES - 1))
                    o_t = outp.tile([128, KSTRIPE], f32, tag="o_t")
                    nc.any.tensor_copy(out=o_t[:], in_=pso[:])
                    nc.sync.dma_start(
                        out=out[qt * 128:(qt + 1) * 128, kslice], in_=o_t[:])
    nc.compile()
    return nc


def kernel(q: np.ndarray, k: np.ndarray, embed: np.ndarray) -> np.ndarray:
    from concourse.bass_utils import run_bass_kernel_spmd

    if "nc" not in _CACHE:
        _CACHE["nc"] = _build()
    nc = _CACHE["nc"]

    e = np.ascontiguousarray(embed[:Q], dtype=np.float32)
    in_maps = [
        {
            "q": np.ascontiguousarray(q[b], dtype=np.float32),
            "k": np.ascontiguousarray(k[b], dtype=np.float32),
            "e": e,
        }
        for b in range(B)
    ]
    res = run_bass_kernel_spmd(nc, in_maps, core_ids=list(range(B)), trace=TRACE)
    _CACHE["last_result"] = res
    return np.stack([res.results[b]["out"] for b in range(B)])


# revision 12
# speedup vs baseline: 1.0424x; 1.0016x over previous
"""AbsoluteLearnedPE kernel v4: stripe-major, fully-resident qT/eT,
streamed keT stripes with the k+e add fused into transpose eviction.

Prologue: only q and e row-blocks (DMA + fp32 PE transpose + fp32r
eviction into single big tiles — no slot recycling, no deadlock).
Stripe phases: k row-blocks stream in; each k-transpose evicts through
a DVE add (psum kT + eT slice -> fp32r keT stripe), then 16 matmul
groups per stripe run while the next stripe's k transposes overlap.
"""

import numpy as np

B, Q, K, D = 8, 2048, 2048, 1024
DTILES = D // 128
RBLK = Q // 128
QT = Q // 128
KSTRIPE = 512
KS = K // KSTRIPE
RB_PER_STRIPE = KSTRIPE // 128   # 4

_CACHE = {}
TRACE = False


def _build():
    from concourse import bacc
    import concourse.mybir as mybir
    import concourse.tile as tile
    from concourse.masks import make_identity

    f32 = mybir.dt.float32
    f32r = mybir.dt.float32r

    nc = bacc.Bacc("TRN2", target_bir_lowering=False, debug=False, num_devices=B)
    q = nc.dram_tensor("q", [Q, D], f32, kind="ExternalInput").ap()
    k = nc.dram_tensor("k", [K, D], f32, kind="ExternalInput").ap()
    e = nc.dram_tensor("e", [Q, D], f32, kind="ExternalInput").ap()
    out = nc.dram_tensor("out", [Q, K], f32, kind="ExternalOutput").ap()

    with tile.TileContext(nc) as tc:
        with tc.tile_pool(name="const", bufs=1) as const, \
             tc.tile_pool(name="nat", bufs=2) as nat, \
             tc.tile_pool(name="big", bufs=1) as big, \
             tc.tile_pool(name="kst", bufs=2) as kst, \
             tc.tile_pool(name="outp", bufs=2) as outp, \
             tc.tile_pool(name="tps", bufs=4, space="PSUM") as tps, \
             tc.tile_pool(name="mps", bufs=4, space="PSUM") as mps:

            ident = const.tile([128, 128], f32)
            make_identity(nc, ident)

            # Fully-resident transposed operands (single allocations).
            qT = big.tile([128, DTILES, Q], f32r, tag="qT")
            eT = big.tile([128, DTILES, Q], f32r, tag="eT")

            # Prologue: q + e row-blocks -> qT / eT.
            for r in range(RBLK):
                rs = slice(r * 128, (r + 1) * 128)
                q_nat = nat.tile([128, D], f32, tag="q_nat")
                e_nat = nat.tile([128, D], f32, tag="e_nat")
                nc.sync.dma_start(out=q_nat[:], in_=q[rs, :])
                nc.sync.dma_start(out=e_nat[:], in_=e[rs, :])
                for d in range(DTILES):
                    ds_ = slice(d * 128, (d + 1) * 128)
                    ps = tps.tile([128, 128], f32, tag="tps")
                    nc.tensor.transpose(ps[:], q_nat[:, ds_], ident[:])
                    nc.any.tensor_copy(out=qT[:, d, rs], in_=ps[:])
                    ps2 = tps.tile([128, 128], f32, tag="tps")
                    nc.tensor.transpose(ps2[:], e_nat[:, ds_], ident[:])
                    nc.any.tensor_copy(out=eT[:, d, rs], in_=ps2[:])

            # Stripe phases.
            for ks in range(KS):
                kslice = slice(ks * KSTRIPE, (ks + 1) * KSTRIPE)
                keT = kst.tile([128, DTILES, KSTRIPE], f32r, tag="keT")
                for rr in range(RB_PER_STRIPE):
                    r = ks * RB_PER_STRIPE + rr
                    rs = slice(r * 128, (r + 1) * 128)
                    ws = slice(rr * 128, (rr + 1) * 128)
                    k_nat = nat.tile([128, D], f32, tag="k_nat")
                    nc.sync.dma_start(out=k_nat[:], in_=k[rs, :])
                    for d in range(DTILES):
                        ds_ = slice(d * 128, (d + 1) * 128)
                        ps = tps.tile([128, 128], f32, tag="tps")
                        nc.tensor.transpose(ps[:], k_nat[:, ds_], ident[:])
                        # fused k+e: keT = kT (exact, from PSUM) + eT (fp32r)
                        nc.vector.tensor_add(keT[:, d, ws], ps[:], eT[:, d, rs])
                for qt in range(QT):
                    qs = slice(qt * 128, (qt + 1) * 128)
                    pso = mps.tile([128, KSTRIPE], f32, tag="mps")
                    for d in range(DTILES):
                        nc.tensor.matmul(pso[:], qT[:, d, qs], eT[:, d, kslice],
                                         start=(d == 0), stop=False)
                    for d in range(DTILES):
                        nc.tensor.matmul(pso[:], eT[:, d, qs], keT[:, d, :],
                                         start=False, stop=(d == DTILES - 1))
                    o_t = outp.tile([128, KSTRIPE], f32, tag="o_t")
                    nc.any.tensor_copy(out=o_t[:], in_=pso[:])
                    nc.sync.dma_start(out=out[qs, kslice], in_=o_t[:])
    nc.compile()
    return nc


def kernel(q: np.ndarray, k: np.ndarray, embed: np.ndarray) -> np.ndarray:
    from concourse.bass_utils import run_bass_kernel_spmd

    if "nc" not in _CACHE:
        _CACHE["nc"] = _build()
    nc = _CACHE["nc"]

    e = np.ascontiguousarray(embed[:Q], dtype=np.float32)
    in_maps = [
        {
            "q": np.ascontiguousarray(q[b], dtype=np.float32),
            "k": np.ascontiguousarray(k[b], dtype=np.float32),
            "e": e,
        }
        for b in range(B)
    ]
    res = run_bass_kernel_spmd(nc, in_maps, core_ids=list(range(B)), trace=TRACE)
    _CACHE["last_result"] = res
    return np.stack([res.results[b]["out"] for b in range(B)])


# revision 13
# speedup vs baseline: 1.1372x; 1.0909x over previous
"""AbsoluteLearnedPE kernel v4b: stripe-major, fully-resident qT/eT,
streamed keT stripes with the k+e add fused into transpose eviction.

Prologue: only q and e row-blocks (DMA + fp32 PE transpose + fp32r
eviction into single big tiles — no slot recycling, no deadlock).
Stripe phases: k row-blocks stream in; each k-transpose evicts through
a DVE add (psum kT + eT slice -> fp32r keT stripe), then 16 matmul
groups per stripe run while the next stripe's k transposes overlap.
"""

import numpy as np

B, Q, K, D = 8, 2048, 2048, 1024
DTILES = D // 128
RBLK = Q // 128
QT = Q // 128
KSTRIPE = 512
KS = K // KSTRIPE
RB_PER_STRIPE = KSTRIPE // 128   # 4

_CACHE = {}
TRACE = False


def _build():
    from concourse import bacc
    import concourse.mybir as mybir
    import concourse.tile as tile
    from concourse.masks import make_identity

    f32 = mybir.dt.float32
    f32r = mybir.dt.float32r

    nc = bacc.Bacc("TRN2", target_bir_lowering=False, debug=False, num_devices=B)
    q = nc.dram_tensor("q", [Q, D], f32, kind="ExternalInput").ap()
    k = nc.dram_tensor("k", [K, D], f32, kind="ExternalInput").ap()
    e = nc.dram_tensor("e", [Q, D], f32, kind="ExternalInput").ap()
    out = nc.dram_tensor("out", [Q, K], f32, kind="ExternalOutput").ap()

    with tile.TileContext(nc) as tc:
        with tc.tile_pool(name="const", bufs=1) as const, \
             tc.tile_pool(name="nat", bufs=2) as nat, \
             tc.tile_pool(name="big", bufs=1) as big, \
             tc.tile_pool(name="kst", bufs=2) as kst, \
             tc.tile_pool(name="outp", bufs=2) as outp, \
             tc.tile_pool(name="tps", bufs=4, space="PSUM") as tps, \
             tc.tile_pool(name="mps", bufs=4, space="PSUM") as mps:

            ident = const.tile([128, 128], f32)
            make_identity(nc, ident)

            # Fully-resident transposed operands (single allocations).
            qT = big.tile([128, DTILES, Q], f32r, tag="qT")
            eT = big.tile([128, DTILES, Q], f32r, tag="eT")

            keT_tiles = {}

            def ensure_keT(ks):
                if ks not in keT_tiles:
                    keT = kst.tile([128, DTILES, KSTRIPE], f32r, tag="keT")
                    keT_tiles[ks] = keT
                return keT_tiles[ks]

            def emit_kblock(r):
                # k row-block r -> transposes + fused (+eT) eviction into its
                # stripe's keT tile. Requires eT row-block r already emitted.
                ks, rr = divmod(r, RB_PER_STRIPE)
                keT = ensure_keT(ks)
                rs = slice(r * 128, (r + 1) * 128)
                ws = slice(rr * 128, (rr + 1) * 128)
                k_nat = nat.tile([128, D], f32, tag="k_nat")
                nc.sync.dma_start(out=k_nat[:], in_=k[rs, :])
                for d in range(DTILES):
                    ds_ = slice(d * 128, (d + 1) * 128)
                    ps = tps.tile([128, 128], f32, tag="tps")
                    nc.tensor.transpose(ps[:], k_nat[:, ds_], ident[:])
                    # fused k+e: keT = kT (exact, from PSUM) + eT (fp32r)
                    nc.vector.tensor_add(keT[:, d, ws], ps[:], eT[:, d, rs])

            # Prologue: q + e row-blocks -> qT / eT, with stripe-0/1 k-blocks
            # woven in so the first matmul groups unlock mid-prologue.
            for r in range(RBLK):
                rs = slice(r * 128, (r + 1) * 128)
                q_nat = nat.tile([128, D], f32, tag="q_nat")
                e_nat = nat.tile([128, D], f32, tag="e_nat")
                nc.sync.dma_start(out=q_nat[:], in_=q[rs, :])
                nc.sync.dma_start(out=e_nat[:], in_=e[rs, :])
                for d in range(DTILES):
                    ds_ = slice(d * 128, (d + 1) * 128)
                    ps = tps.tile([128, 128], f32, tag="tps")
                    nc.tensor.transpose(ps[:], q_nat[:, ds_], ident[:])
                    nc.any.tensor_copy(out=qT[:, d, rs], in_=ps[:])
                    ps2 = tps.tile([128, 128], f32, tag="tps")
                    nc.tensor.transpose(ps2[:], e_nat[:, ds_], ident[:])
                    nc.any.tensor_copy(out=eT[:, d, rs], in_=ps2[:])
                if 1 <= r <= 8:
                    emit_kblock(r - 1)

            # Stripe phases.
            for ks in range(KS):
                kslice = slice(ks * KSTRIPE, (ks + 1) * KSTRIPE)
                keT = ensure_keT(ks)
                for rr in range(RB_PER_STRIPE):
                    r = ks * RB_PER_STRIPE + rr
                    if r > 7:
                        emit_kblock(r)
                for qt in range(QT):
                    qs = slice(qt * 128, (qt + 1) * 128)
                    pso = mps.tile([128, KSTRIPE], f32, tag="mps")
                    for d in range(DTILES):
                        nc.tensor.matmul(pso[:], qT[:, d, qs], eT[:, d, kslice],
                                         start=(d == 0), stop=False)
                    for d in range(DTILES):
                        nc.tensor.matmul(pso[:], eT[:, d, qs], keT[:, d, :],
                                         start=False, stop=(d == DTILES - 1))
                    o_t = outp.tile([128, KSTRIPE], f32, tag="o_t")
                    nc.any.tensor_copy(out=o_t[:], in_=pso[:])
                    nc.sync.dma_start(out=out[qs, kslice], in_=o_t[:])
    nc.compile()
    return nc


def kernel(q: np.ndarray, k: np.ndarray, embed: np.ndarray) -> np.ndarray:
    from concourse.bass_utils import run_bass_kernel_spmd

    if "nc" not in _CACHE:
        _CACHE["nc"] = _build()
    nc = _CACHE["nc"]

    e = np.ascontiguousarray(embed[:Q], dtype=np.float32)
    in_maps = [
        {
            "q": np.ascontiguousarray(q[b], dtype=np.float32),
            "k": np.ascontiguousarray(k[b], dtype=np.float32),
            "e": e,
        }
        for b in range(B)
    ]
    res = run_bass_kernel_spmd(nc, in_maps, core_ids=list(range(B)), trace=TRACE)
    _CACHE["last_result"] = res
    return np.stack([res.results[b]["out"] for b in range(B)])
